# revision 3
# baseline (speedup 1.0000x reference)
"""GatingAttention (AlphaFold-style) Trainium2 kernel, bf16 pipelined.

B=256 batches sharded across 8 NeuronCores (32/core, 16 pairs), with a
3-deep software pipeline across pairs: DMA(p+2) | proj(p+1) | logits+
exp(p) | attention+output(p-1), so PE/ACT/DVE all see steady work.

All matmul operands are bf16 (fp32 PSUM accumulation).  The
nonbatched_bias is accumulated into the logits PSUM by identity
matmuls (start=True) before the QK matmuls (start=False) -- no
elementwise nb work at all.  The sigmoid gate is computed as
tanh((x+gb)/2) so the whole kernel stays on ACT's exp_and_others
table set (exp+tanh, zero table reloads); sigmoid/denominator are
fused via u=(tanh+1)*(0.5/denom) with the 0.5 pre-folded into the
denominator matmul's lhsT (2*exp(bias) columns).

Per-(head-group) gating runs at [128, 512] (both batches at once):
recip + scalar_tensor_tensor + mul = just 3 vector ops.  The output
projection is DMA'd straight from PSUM; the output bias is added on
host during unshard.
"""
import numpy as np
import ml_dtypes
from contextlib import ExitStack

import concourse.bass as bass
import concourse.tile as tile
from concourse import bacc, mybir
from concourse.bass_utils import run_bass_kernel_spmd

dt = mybir.dt
AF = mybir.ActivationFunctionType

N_CORES = 8
B, S, A, M, H, OUT = 256, 256, 256, 256, 8, 256
KD = VD = 32
BC = B // N_CORES          # 32 batches per core
NPAIR = BC // 2            # 16 pairs

_CACHE = {}


def build_nc(npair=NPAIR, num_devices=N_CORES, reps=1, nb_on_pe=True,
             exg_bufs=18, io_bufs=3, stt_pool=False, fine=True,
             dup_act=False, dup_dve=False, dup_pe=False):
    f32, f32r, bf16 = dt.float32, dt.float32r, dt.bfloat16
    nc = bacc.Bacc("TRN2", target_bir_lowering=False, debug=False,
                   num_devices=num_devices)

    def inp(name, shape, d):
        return nc.dram_tensor(name, shape, d, kind="ExternalInput").ap()

    qdT = inp("qdT", [npair, 128, 1024], bf16)   # col = kc*512 + b*256 + s
    mdT = inp("mdT", [npair, 128, 1024], bf16)
    wq = inp("wq", [2, 128, 256], bf16)
    wk = inp("wk", [2, 128, 256], bf16)
    wg = inp("wg", [2, 128, 256], bf16)
    wv = inp("wv", [2, 128, 256], bf16)
    wo = inp("wo", [2, 128, 256], bf16)
    expb1 = inp("expb1", [128, npair * 4], f32)        # col = p*4 + c*2 + b
    expb32 = inp("expb32", [128, npair * 128], bf16)   # 2*exp(bias), x32
    if nb_on_pe:
        pnb = inp("pnb", [4, 2, 128, 1024], f32r)      # head-paired nb
        ident = inp("ident", [128, 128], f32r)
    else:
        expnb = inp("expnb", [4, 2, 128, 1024], bf16)  # head-paired exp(nb)
    gb = inp("gb", [2, 128, 1], f32)                   # 0.5 * gating_b
    outT = nc.dram_tensor("outT", [npair, 2, 128, 512], bf16,
                          kind="ExternalOutput").ap()

    with tile.TileContext(nc) as tc, ExitStack() as ctx:
        const = ctx.enter_context(tc.tile_pool(name="const", bufs=1))

        def resident(ap, d, tag):
            t = const.tile(list(ap.shape), d, tag=tag)
            nc.sync.dma_start(t[:], ap)
            return t

        wq_t = [resident(wq[c], bf16, f"wq{c}") for c in range(2)]
        wk_t = [resident(wk[c], bf16, f"wk{c}") for c in range(2)]
        wg_t = [resident(wg[c], bf16, f"wg{c}") for c in range(2)]
        wv_t = [resident(wv[c], bf16, f"wv{c}") for c in range(2)]
        wo_t = [resident(wo[c], bf16, f"wo{c}") for c in range(2)]
        expb1_t = resident(expb1, f32, "expb1")
        expb32_t = resident(expb32, bf16, "expb32")
        if nb_on_pe:
            pnb_t = [[resident(pnb[hh, c], f32r, f"pnb{hh}_{c}")
                      for c in range(2)] for hh in range(4)]
            ident_t = resident(ident, f32r, "ident")
        else:
            expnb_t = [[resident(expnb[hh, c], bf16, f"expnb{hh}_{c}")
                        for c in range(2)] for hh in range(4)]
        gb_t = [resident(gb[c], f32, f"gb{c}") for c in range(2)]

        io = ctx.enter_context(tc.tile_pool(name="io", bufs=io_bufs))
        proj = ctx.enter_context(tc.tile_pool(name="proj", bufs=3))
        vpool = ctx.enter_context(tc.tile_pool(name="vpool", bufs=3))
        exgp = ctx.enter_context(tc.tile_pool(name="exgp", bufs=exg_bufs))
        gwap = ctx.enter_context(tc.tile_pool(name="gwap", bufs=2))
        smallp = ctx.enter_context(tc.tile_pool(name="smallp", bufs=3))

        pp = ctx.enter_context(tc.tile_pool(name="pp", bufs=2, space="PSUM"))
        lgp = ctx.enter_context(tc.tile_pool(name="lgp", bufs=2, space="PSUM"))
        wdp = ctx.enter_context(tc.tile_pool(name="wdp", bufs=2, space="PSUM"))

        st = {}  # per-pair pipeline state

        def stage_dma(p):
            qd = io.tile([128, 1024], bf16, tag="qd")
            nc.sync.dma_start(qd[:], qdT[p])
            md = io.tile([128, 1024], bf16, tag="md")
            nc.sync.dma_start(md[:], mdT[p])
            st[p] = dict(qd=qd, md=md)

        def chunks_a(p):
            s = st[p]
            qd, md = s["qd"], s["md"]
            s.update(qT2=[None, None], kT2=[None, None], gate=[None, None],
                     vp=[None, None])

            def mk_qk(w_t, src, dstname, tagn, mt):
                def emit():
                    ps = pp.tile([128, 512], f32, tag="projps")
                    for kc in range(2):
                        nc.tensor.matmul(
                            ps[:], w_t[kc][:, mt * 128:(mt + 1) * 128],
                            src[:, kc * 512:(kc + 1) * 512],
                            start=kc == 0, stop=kc == 1)
                    t = proj.tile([128, 512], bf16, tag=f"{tagn}{mt}")
                    nc.vector.tensor_copy(t[:], ps[:])
                    s[dstname][mt] = t
                return emit

            def mk_gate(mt):
                def emit():
                    ps = pp.tile([128, 512], f32, tag="projps")
                    for kc in range(2):
                        nc.tensor.matmul(
                            ps[:], wg_t[kc][:, mt * 128:(mt + 1) * 128],
                            qd[:, kc * 512:(kc + 1) * 512],
                            start=kc == 0, stop=kc == 1)
                    t = proj.tile([128, 512], bf16, tag=f"gate{mt}")
                    nc.scalar.activation(t[:], ps[:], AF.Tanh,
                                         bias=gb_t[mt][:, 0:1], scale=0.5)
                    s["gate"][mt] = t
                return emit

            def mk_v(b):
                def emit():
                    ps = pp.tile([128, 512], f32, tag="projps")
                    for c in range(2):
                        for kc in range(2):
                            nc.tensor.matmul(
                                ps[:, c * 256:(c + 1) * 256],
                                md[:, kc * 512 + b * 256 + c * 128:
                                   kc * 512 + b * 256 + (c + 1) * 128],
                                wv_t[kc][:], start=kc == 0, stop=kc == 1)
                    t = vpool.tile([128, 512], bf16, tag=f"vpx{b}")
                    for c in range(2):
                        col = p * 4 + c * 2 + b
                        nc.vector.tensor_scalar_mul(
                            t[:, c * 256:(c + 1) * 256],
                            ps[:, c * 256:(c + 1) * 256],
                            expb1_t[:, col:col + 1])
                    s["vp"][b] = t
                return emit

            return [mk_qk(wq_t, qd, "qT2", "qT", 0),
                    mk_qk(wq_t, qd, "qT2", "qT", 1),
                    mk_qk(wk_t, md, "kT2", "kT", 0),
                    mk_qk(wk_t, md, "kT2", "kT", 1),
                    mk_gate(0), mk_gate(1), mk_v(0), mk_v(1)]

        def stage_a(p):
            for f in chunks_a(p):
                f()

        def chunks_b(p):
            s = st[p]
            s["exg"] = [[None, None] for _ in range(4)]

            def mk_tile(c, hh, nmul):
                def emit():
                    qT2, kT2 = s["qT2"], s["kT2"]
                    exg = s["exg"]
                    lg = lgp.tile([128, 1024], f32, tag="lg")
                    if nb_on_pe:
                        for half in range(2):
                            nc.tensor.matmul(
                                lg[:, half * 512:(half + 1) * 512],
                                ident_t[:],
                                pnb_t[hh][c][:, half * 512:(half + 1) * 512],
                                start=True, stop=False, skip_group_check=True)
                    for par in range(2):
                        h = 2 * hh + par
                        ht, hr = h // 4, h % 4
                        for b in range(2):
                            nc.tensor.matmul(
                                lg[:, par * 512 + b * 256:
                                   par * 512 + (b + 1) * 256],
                                kT2[ht][hr * 32:(hr + 1) * 32,
                                        b * 256 + c * 128:
                                        b * 256 + (c + 1) * 128],
                                qT2[ht][hr * 32:(hr + 1) * 32,
                                        b * 256:(b + 1) * 256],
                                start=not nb_on_pe, stop=True,
                                tile_position=(hr * 32, 0),
                                skip_group_check=nb_on_pe)
                    e = exgp.tile([128, 1024], bf16, tag="exg")
                    nc.scalar.activation(e[:], lg[:], AF.Exp)
                    if dup_act:
                        ed = exgp.tile([128, 1024], bf16, tag="exgdup")
                        nc.scalar.activation(ed[:], lg[:], AF.Exp)
                    if dup_dve:
                        ed2 = exgp.tile([128, 1024], bf16, tag="dvedup")
                        nc.vector.tensor_mul(ed2[:], e[:], e[:])
                    if dup_pe:
                        # probe: re-run the QK matmuls into the same (already
                        # consumed) psum tile to add pure PE load
                        for par in range(2):
                            h = 2 * hh + par
                            ht, hr = h // 4, h % 4
                            for b in range(2):
                                nc.tensor.matmul(
                                    lg[:, par * 512 + b * 256:
                                       par * 512 + (b + 1) * 256],
                                    s["kT2"][ht][hr * 32:(hr + 1) * 32,
                                                 b * 256 + c * 128:
                                                 b * 256 + (c + 1) * 128],
                                    s["qT2"][ht][hr * 32:(hr + 1) * 32,
                                                 b * 256:(b + 1) * 256],
                                    start=True, stop=True,
                                    tile_position=(hr * 32, 0))
                    if not nb_on_pe:
                        eng = nc.gpsimd if nmul % 4 == 3 else nc.vector
                        eng.tensor_mul(e[:], e[:], expnb_t[hh][c][:])
                    exg[hh][c] = e
                return emit

            return [mk_tile(c, hh, c * 4 + hh)
                    for c in range(2) for hh in range(4)]

        def stage_b(p):
            for f in chunks_b(p):
                f()

        def chunks_c(p):
            s = st[p]
            gwaT = []
            for hg in range(2):
                gw_tile = gwap.tile([128, 512], bf16, tag=f"gwa{hg}")
                gwaT.append(gw_tile)
            s["gwaT"] = gwaT

            def mk_hgb(hg, b):
                def emit():
                    vp, exg, gate = s["vp"], s["exg"], s["gate"]
                    wd = wdp.tile([128, 512], f32, tag="wd")
                    for h4 in range(4):
                        h = hg * 4 + h4
                        hh, par = h // 2, h % 2
                        for c in range(2):
                            nc.tensor.matmul(
                                wd[h4 * 32:(h4 + 1) * 32, 0:256],
                                vp[b][:, c * 256 + h * 32:
                                   c * 256 + (h + 1) * 32],
                                exg[hh][c][:, par * 512 + b * 256:
                                           par * 512 + (b + 1) * 256],
                                start=c == 0, stop=c == 1,
                                tile_position=(0, h4 * 32))
                    for h4 in range(4):
                        h = hg * 4 + h4
                        hh, par = h // 2, h % 2
                        for c in range(2):
                            col = (p * 4 + c * 2 + b) * 32
                            nc.tensor.matmul(
                                wd[h4 * 32:(h4 + 1) * 32, 256:512],
                                expb32_t[:, col:col + 32],
                                exg[hh][c][:, par * 512 + b * 256:
                                           par * 512 + (b + 1) * 256],
                                start=c == 0, stop=c == 1,
                                tile_position=(0, h4 * 32))
                    rec = smallp.tile([128, 256], f32, tag="rec")
                    nc.vector.reciprocal_approx_fast(rec[:], wd[:, 256:512])
                    with nc.allow_low_precision(reason="softmax weights"):
                        u = smallp.tile([128, 256], bf16, tag="u")
                        nc.vector.scalar_tensor_tensor(
                            u[:], gate[hg][:, b * 256:(b + 1) * 256], 1.0,
                            rec[:], mybir.AluOpType.add, mybir.AluOpType.mult)
                    nc.vector.tensor_mul(gwaT[hg][:, b * 256:(b + 1) * 256],
                                         wd[:, 0:256], u[:])
                return emit

            def mk_out(mt):
                def emit():
                    ps = pp.tile([128, 512], f32, tag="projps")
                    for kc in range(2):
                        nc.tensor.matmul(
                            ps[:], wo_t[kc][:, mt * 128:(mt + 1) * 128],
                            s["gwaT"][kc][:], start=kc == 0, stop=kc == 1)
                    o = smallp.tile([128, 512], bf16, tag=f"out{mt}")
                    nc.vector.tensor_copy(o[:], ps[:])
                    nc.sync.dma_start(outT[p, mt], o[:])
                    if mt == 1:
                        del st[p]
                return emit

            return [mk_hgb(0, 0), mk_hgb(0, 1), mk_hgb(1, 0), mk_hgb(1, 1),
                    mk_out(0), mk_out(1)]

        def stage_c(p):
            for f in chunks_c(p):
                f()

        rep_ctx = tc.For_i(0, reps, 1) if reps > 1 else None
        if rep_ctx is not None:
            ctx.enter_context(rep_ctx)
        # software pipeline: DMA(p+2) | A(p+1) | B(p) | C(p-1)
        for step in range(npair + 2):
            has_a = step < npair
            has_b = 1 <= step < npair + 1
            has_c = step >= 2
            if has_a:
                if step == 0:
                    stage_dma(0)
                else:
                    if step + 1 < npair:
                        stage_dma(step + 1)
            if step == 0 and npair > 1:
                pass
            if not fine:
                if has_a:
                    stage_a(step)
                    if step == 0 and npair > 1:
                        stage_dma(1)
                if has_b:
                    stage_b(step - 1)
                if has_c:
                    stage_c(step - 2)
            else:
                ca = chunks_a(step) if has_a else []
                cb = chunks_b(step - 1) if has_b else []
                cc = chunks_c(step - 2) if has_c else []
                qs = [list(cb), list(ca), list(cc)]
                while any(qs):
                    for q in qs:
                        if q:
                            q.pop(0)()
                if step == 0 and npair > 1:
                    stage_dma(1)

    nc.compile()
    return nc


def prep_shared(query_w, key_w, value_w, gating_w, gating_b, output_w,
                output_b, nonbatched_bias):
    f32 = np.float32
    bf16 = ml_dtypes.bfloat16
    wq = (query_w.reshape(A, H * KD) * KD ** -0.5).astype(bf16).reshape(2, 128, 256)
    wk = key_w.reshape(M, H * KD).astype(bf16).reshape(2, 128, 256)
    wv = value_w.reshape(M, H * VD).astype(bf16).reshape(2, 128, 256)
    wg = gating_w.reshape(A, H * VD).astype(bf16).reshape(2, 128, 256)
    wo = output_w.reshape(H * VD, OUT).astype(bf16).reshape(2, 128, 256)

    def btile(x):
        # [H, sk, sq] -> [hh, c, 128, 1024], cols = par*512 + b*256 + sq
        x = np.ascontiguousarray(x.transpose(0, 2, 1)).reshape(H, 2, 128, 256)
        x = np.tile(x, (1, 1, 1, 2))
        x = x.reshape(4, 2, 2, 128, 512).transpose(0, 2, 3, 1, 4)
        return np.ascontiguousarray(x.reshape(4, 2, 128, 1024))

    nbf = nonbatched_bias.astype(f32)
    gbv = (0.5 * gating_b.reshape(H * VD)).astype(f32).reshape(2, 128, 1)
    return dict(wq=np.ascontiguousarray(wq), wk=np.ascontiguousarray(wk),
                wv=np.ascontiguousarray(wv), wg=np.ascontiguousarray(wg),
                wo=np.ascontiguousarray(wo),
                expnb=btile(np.exp(nbf)).astype(bf16),
                pnb=btile(nbf).astype(f32),
                ident=np.eye(128, dtype=f32),
                gb=np.ascontiguousarray(gbv))


def prep_core(q_c, m_c, bias_c, npair=NPAIR):
    """q_c, m_c: [2*npair, S, F]; bias_c: [2*npair, S]."""
    f32 = np.float32
    bf16 = ml_dtypes.bfloat16

    def tr(x):
        x = x.transpose(0, 2, 1)                       # [nb, f, s]
        x = x.reshape(npair, 2, 2, 128, 256)           # [p, b, kc, row, s]
        x = x.transpose(0, 3, 2, 1, 4)                 # [p, row, kc, b, s]
        return np.ascontiguousarray(
            x.reshape(npair, 128, 1024)).astype(bf16)

    eb = np.exp(bias_c.astype(f32))                    # [nb, sk]
    e1 = eb.reshape(npair, 2, 2, 128).transpose(3, 0, 2, 1)  # [128, p, c, b]
    e1 = np.ascontiguousarray(e1.reshape(128, npair * 4))
    e32 = np.ascontiguousarray(np.repeat(2.0 * e1, 32, axis=1)).astype(bf16)
    return dict(qdT=tr(q_c), mdT=tr(m_c), expb1=e1, expb32=e32)


def unshard_out(oT, output_b, npair=NPAIR):
    """oT: [npair, 2, 128, 512] f32 -> [2*npair, S, OUT] f32 (+bias)."""
    y = np.asarray(oT).astype(np.float32)
    y = y.reshape(npair, 2, 128, 2, 256)               # [p, mt, op, b, s]
    y = y.transpose(0, 3, 1, 2, 4)                     # [p, b, mt, op, s]
    y = y.reshape(npair * 2, 256, 256)                 # [nb, o, s]
    y = np.ascontiguousarray(y.transpose(0, 2, 1))     # [nb, s, o]
    y += output_b.astype(np.float32)[None, None, :]
    return y


def kernel(q_data, m_data, bias, nonbatched_bias, query_w, key_w, value_w,
           gating_w, gating_b, output_w, output_b):
    if "nc" not in _CACHE:
        _CACHE["nc"] = build_nc()
    nc = _CACHE["nc"]

    shared = prep_shared(np.asarray(query_w), np.asarray(key_w),
                         np.asarray(value_w), np.asarray(gating_w),
                         np.asarray(gating_b), np.asarray(output_w),
                         np.asarray(output_b), np.asarray(nonbatched_bias))
    q_data = np.asarray(q_data)
    m_data = np.asarray(m_data)
    output_b = np.asarray(output_b)
    bias2 = np.asarray(bias).reshape(B, S)

    in_maps = []
    for c in range(N_CORES):
        sl = slice(c * BC, (c + 1) * BC)
        im = dict(shared)
        im.update(prep_core(q_data[sl], m_data[sl], bias2[sl]))
        in_maps.append(im)

    res = run_bass_kernel_spmd(nc, in_maps, list(range(N_CORES)))
    outs = [unshard_out(res.results[c]["outT"], output_b)
            for c in range(N_CORES)]
    return np.concatenate(outs, axis=0).astype(np.float32)

